# revision 16
# baseline (speedup 1.0000x reference)
"""Trainium2 Bass kernel for nn_BasicGRUBlock: 2-layer GRU block.

  x = y + z; h1 = GRU0(x); h2 = GRU1(h1); out = y + h2 @ W_lin.T + b_lin

Sharding: data-parallel over batch across 8 cores (8 sequences/core).

Gate-major design: all per-step tensors live as [gates/hidden on 128
partitions, batch on free axis].  Recurrent matmuls keep the weights
stationary (bf16, 128-col tiles -> compiler FWL) and stream h^T [128, 8];
gates emerge in PSUM as [128, chunks, 8], so every element-wise op is a
wide 128-partition op with 16-32 free elements, and the hidden update
lands directly in the layout the next matmul consumes - no transposes in
the recurrent chain.

Pipeline per group of GRP=16 steps (body g):
  [L0 step k of group g ; L1 step k of group g-1] x 16   (interleaved)
  gx1(g):   W_ih1 @ H1(g) group matmuls -> G1/N1
  final(g-1): out = y + W_lin @ H2(g-1) + b_lin -> DMA
  bulk0(g+1): DMA y,z; x=y+z; x^T; W_ih0 @ x^T -> G0/N0
L1 lags L0 by one group; double-buffered via even/odd parity tiles.
"""

import sys

sys.path.insert(0, "/opt/trn_rl_repo")

import numpy as np

import concourse.bass as bass
import concourse.bacc as bacc_mod
import concourse.mybir as mybir
from concourse.bass import ds
from concourse.tile import TileContext

B, T_FULL, I, H, G = 64, 4096, 64, 256, 768
NCORES = 8
BL = B // NCORES  # 8 sequences per core
GRP = 16          # time steps per group
C = GRP * BL      # 128 columns per group (col = t*8 + b)
NJ = 6            # gate chunks of 128 (r: 0-1, z: 2-3, n: 4-5)
NI = 2            # hidden chunks of 128
F32 = mybir.dt.float32
F32R = mybir.dt.float32r
BF16 = mybir.dt.bfloat16

SIG = mybir.ActivationFunctionType.Sigmoid
TANH = mybir.ActivationFunctionType.Tanh
MULT = mybir.AluOpType.mult
ADD = mybir.AluOpType.add
SUB = mybir.AluOpType.subtract


def _r(ap):
    return ap.bitcast(F32R)


def build_nc(T=T_FULL, unroll_all=False, debug=False):
    """unroll_all=True builds a fully python-unrolled program (for sim)."""
    nc = bacc_mod.Bacc()

    NG = T // GRP
    RPAD_IN = (NG + 2) * C    # y/z rows incl. 2 pad groups
    RPAD_OUT = (NG + 1) * C   # out rows incl. 1 pad group (dropped by host)

    y_d = nc.declare_dram_parameter("y", [RPAD_IN, I], F32R, isOutput=False)
    z_d = nc.declare_dram_parameter("z", [RPAD_IN, I], F32, isOutput=False)
    whh0_d = nc.declare_dram_parameter("whh0", [128, NJ, NI, 128], BF16,
                                       isOutput=False)
    whh1_d = nc.declare_dram_parameter("whh1", [128, NJ, NI, 128], BF16,
                                       isOutput=False)
    wih1_d = nc.declare_dram_parameter("wih1", [128, NJ, NI, 128], BF16,
                                       isOutput=False)
    wih0a_d = nc.declare_dram_parameter("wih0a", [I + 1, NJ, 128], BF16,
                                        isOutput=False)
    wlin_d = nc.declare_dram_parameter("wlin", [128, NI, I], BF16,
                                       isOutput=False)
    bias1_d = nc.declare_dram_parameter("bias1", [1, NJ, 128], BF16,
                                        isOutput=False)
    blin_d = nc.declare_dram_parameter("blin", [1, I], BF16, isOutput=False)
    bc0_d = nc.declare_dram_parameter("bc0", [128, 2, BL], BF16,
                                      isOutput=False)
    bc1_d = nc.declare_dram_parameter("bc1", [128, 2, BL], BF16,
                                      isOutput=False)
    eyef_d = nc.declare_dram_parameter("eyef", [128, 128], F32,
                                       isOutput=False)
    eyeb_d = nc.declare_dram_parameter("eyeb", [128, 128], BF16,
                                       isOutput=False)
    eyer_d = nc.declare_dram_parameter("eyer", [128, 128], F32R,
                                       isOutput=False)
    out_d = nc.declare_dram_parameter("out", [RPAD_OUT, I], F32,
                                      isOutput=True)
    h1dbg_d = h2dbg_d = None
    if debug:
        h1dbg_d = nc.declare_dram_parameter("h1dbg", [RPAD_OUT, NI, 128],
                                            BF16, isOutput=True)
        h2dbg_d = nc.declare_dram_parameter("h2dbg", [RPAD_OUT, NI, 128],
                                            BF16, isOutput=True)

    with TileContext(nc) as tc:
        with (
            tc.tile_pool(name="wpool", bufs=1) as wpool,
            tc.tile_pool(name="gatep", bufs=3) as gatep,
            tc.tile_pool(name="iop", bufs=2) as iop,
            tc.tile_pool(name="ps_a0", bufs=1, space="PSUM") as ps_a0,
            tc.tile_pool(name="ps_n0", bufs=1, space="PSUM") as ps_n0,
            tc.tile_pool(name="ps_a1", bufs=1, space="PSUM") as ps_a1,
            tc.tile_pool(name="ps_n1", bufs=1, space="PSUM") as ps_n1,
            tc.tile_pool(name="ps_gx", bufs=2, space="PSUM") as ps_gx,
            tc.tile_pool(name="ps_tp", bufs=1, space="PSUM") as ps_tp,
            tc.tile_pool(name="ps_fin", bufs=1, space="PSUM") as ps_fin,
        ):
            # ---- persistent weights / constants ----
            whh0_t = wpool.tile([128, NJ, NI, 128], BF16)
            whh1_t = wpool.tile([128, NJ, NI, 128], BF16)
            wih1_t = wpool.tile([128, NJ, NI, 128], BF16)
            wih0a_t = wpool.tile([I + 1, NJ, 128], BF16)
            wlin_t = wpool.tile([128, NI, I], BF16)
            bias1_t = wpool.tile([1, NJ, 128], BF16)
            blin_t = wpool.tile([1, I], BF16)
            bc0_t = wpool.tile([128, 2, BL], BF16)
            bc1_t = wpool.tile([128, 2, BL], BF16)
            eyef_t = wpool.tile([128, 128], F32)
            eyeb_t = wpool.tile([128, 128], BF16)
            eyer_t = wpool.tile([128, 128], F32R)
            ones1_t = wpool.tile([1, 128], BF16)
            onesf_t = wpool.tile([1, 128], F32)

            nc.sync.dma_start(out=whh0_t, in_=whh0_d[:])
            nc.sync.dma_start(out=whh1_t, in_=whh1_d[:])
            nc.sync.dma_start(out=wih1_t, in_=wih1_d[:])
            nc.sync.dma_start(out=wih0a_t, in_=wih0a_d[:])
            nc.sync.dma_start(out=wlin_t, in_=wlin_d[:])
            nc.sync.dma_start(out=bias1_t, in_=bias1_d[:])
            nc.sync.dma_start(out=blin_t, in_=blin_d[:])
            nc.sync.dma_start(out=bc0_t, in_=bc0_d[:])
            nc.sync.dma_start(out=bc1_t, in_=bc1_d[:])
            nc.sync.dma_start(out=eyef_t, in_=eyef_d[:])
            nc.sync.dma_start(out=eyeb_t, in_=eyeb_d[:])
            nc.sync.dma_start(out=eyer_t, in_=eyer_d[:])
            nc.gpsimd.memset(onesf_t[:], 1.0)
            nc.vector.tensor_copy(ones1_t[:], onesf_t[:])

            # ---- parity-pair state tiles ----
            # per parity p: G/N gx tiles, H group tiles, xTa, y/z input tiles
            G0_ = [wpool.tile([128, 4, GRP, BL], BF16, name=f"G0_{p}")
                   for p in range(2)]
            N0_ = [wpool.tile([128, 2, GRP, BL], BF16, name=f"N0_{p}")
                   for p in range(2)]
            G1_ = [wpool.tile([128, 4, GRP, BL], BF16, name=f"G1_{p}")
                   for p in range(2)]
            N1_ = [wpool.tile([128, 2, GRP, BL], BF16, name=f"N1_{p}")
                   for p in range(2)]
            H1_ = [wpool.tile([128, NI, C], BF16, name=f"H1_{p}")
                   for p in range(2)]
            H2_ = [wpool.tile([128, NI, C], BF16, name=f"H2_{p}")
                   for p in range(2)]
            xTa_ = [wpool.tile([I + 1, 128], BF16, name=f"xTa_{p}")
                    for p in range(2)]
            y_ = [wpool.tile([128, I], F32R, name=f"y_{p}") for p in range(2)]
            z_ = [wpool.tile([128, I], F32, name=f"z_{p}") for p in range(2)]

            # init: ones rows of xTa; zero carries and L1(-1)/final(-1) inputs
            nc.vector.tensor_copy(xTa_[0][I: I + 1, :], onesf_t[:])
            nc.vector.tensor_copy(xTa_[1][I: I + 1, :], onesf_t[:])
            nc.vector.memset(H1_[1][:], 0.0)
            nc.vector.memset(H2_[0][:], 0.0)
            nc.vector.memset(H2_[1][:], 0.0)
            nc.vector.memset(G1_[1][:], 0.0)
            nc.vector.memset(N1_[1][:], 0.0)
            nc.vector.memset(y_[1][:].bitcast(F32), 0.0)

            def gstep(l, k, par):
                """One GRU step. l=0: layer0 group g (parity par);
                l=1: layer1 group g-1 (parity 1-par)."""
                if l == 0:
                    Gt, Nt, Ht = G0_[par], N0_[par], H1_[par]
                    Hprev = H1_[1 - par]
                    W, Bc = whh0_t, bc0_t
                    psA, psN = ps_a0, ps_n0
                else:
                    Gt, Nt, Ht = G1_[1 - par], N1_[1 - par], H2_[1 - par]
                    Hprev = H2_[par]
                    W, Bc = whh1_t, bc1_t
                    psA, psN = ps_a1, ps_n1

                if k == 0:
                    hp = Hprev[:, :, C - BL: C]
                else:
                    hp = Ht[:, :, (k - 1) * BL: k * BL]

                A = psA.tile([128, 4, BL], F32, tag=f"A{l}")
                N = psN.tile([128, 2, BL], F32, tag=f"N{l}")
                for j in range(4):
                    for i in range(NI):
                        nc.tensor.matmul(A[:, j, :], W[:, j, i, :],
                                         hp[:, i, :],
                                         start=(j == 0 and i == 0),
                                         stop=False, skip_group_check=True)
                # fold gx_rz (incl. all rz biases) into PSUM
                nc.tensor.matmul(A[:, :, :], eyeb_t, Gt[:, :, k, :],
                                 start=False, stop=True,
                                 skip_group_check=True)
                for j in range(2):
                    for i in range(NI):
                        nc.tensor.matmul(N[:, j, :], W[:, 4 + j, i, :],
                                         hp[:, i, :],
                                         start=(j == 0 and i == 0),
                                         stop=False, skip_group_check=True)
                # fold b_hh_n into PSUM
                nc.tensor.matmul(N[:, :, :], eyeb_t, Bc[:],
                                 start=False, stop=True,
                                 skip_group_check=True)

                S = gatep.tile([128, 4, BL], F32, tag=f"S{l}")
                nc.scalar.activation(S, A, SIG)
                m = gatep.tile([128, 2, BL], F32, tag=f"m{l}")
                nc.vector.tensor_tensor(m, N, S[:, 0:2, :], MULT)
                t = gatep.tile([128, 2, BL], F32, tag=f"t{l}")
                nc.vector.tensor_tensor(t, m, Nt[:, :, k, :], ADD)
                nt = gatep.tile([128, 2, BL], F32, tag=f"nt{l}")
                nc.scalar.activation(nt, t, TANH)
                # off-chain: zm1 = z - 1 ; c = z * h
                zm1 = gatep.tile([128, 2, BL], F32, tag=f"zm1{l}")
                nc.gpsimd.tensor_scalar(zm1, S[:, 2:4, :], 1.0, None, SUB)
                c = gatep.tile([128, 2, BL], F32, tag=f"c{l}")
                nc.gpsimd.tensor_tensor(c, S[:, 2:4, :], hp.bitcast(BF16),
                                        MULT)
                # chain: tmp = nt * zm1 ; h' = c - tmp
                tmp = gatep.tile([128, 2, BL], F32, tag=f"tmp{l}")
                nc.vector.tensor_tensor(tmp, nt, zm1, MULT)
                nc.vector.tensor_tensor(Ht[:, :, k * BL: (k + 1) * BL],
                                        c, tmp, SUB)

            def gx_copy(j, dst, gps):
                """PSUM->SBUF gx copy, spread across engines by chunk."""
                if j in (0, 1, 2):
                    nc.vector.tensor_copy(dst, gps)
                else:
                    nc.scalar.copy(dst, gps)

            def bulk0(r_y, par):
                """Load y,z for a group into parity `par`, compute x^T and
                gx0 -> G0_[par], N0_[par]."""
                nc.sync.dma_start(out=y_[par], in_=y_d[ds(r_y, C), :])
                nc.sync.dma_start(out=z_[par], in_=z_d[ds(r_y, C), :])
                x_t = iop.tile([128, I], F32, tag="x")
                nc.vector.tensor_tensor(x_t, y_[par].bitcast(F32), z_[par], ADD)
                tp = ps_tp.tile([I, 128], F32, tag="tp")
                nc.tensor.transpose(tp, x_t, eyef_t)
                nc.scalar.copy(xTa_[par][0:I, :], tp)
                for j in range(NJ):
                    gps = ps_gx.tile([128, C], F32, tag="gx")
                    nc.tensor.matmul(gps, wih0a_t[:, j, :], xTa_[par],
                                     start=True, stop=True)
                    if j < 4:
                        dst = G0_[par][:, j, :, :]
                    else:
                        dst = N0_[par][:, j - 4, :, :]
                    gx_copy(j, dst, gps)

            def gx1(par):
                """W_ih1 @ H1(g) -> G1_[par], N1_[par]."""
                for j in range(NJ):
                    gps = ps_gx.tile([128, C], F32, tag="gx")
                    for i in range(NI):
                        nc.tensor.matmul(gps, wih1_t[:, j, i, :],
                                         H1_[par][:, i, :],
                                         start=(i == 0), stop=False)
                    nc.tensor.matmul(gps, bias1_t[:, j, :], ones1_t,
                                     start=False, stop=True)
                    if j < 4:
                        dst = G1_[par][:, j, :, :]
                    else:
                        dst = N1_[par][:, j - 4, :, :]
                    gx_copy(j, dst, gps)

            def final(r_o, par):
                """out rows [r_o, r_o+C) = y + W_lin @ H2(g-1) + b_lin.
                H2/y parity is 1-par (group g-1)."""
                fp = ps_fin.tile([128, I], F32, tag="fin")
                nc.tensor.matmul(fp, H2_[1 - par][:, 0, :], wlin_t[:, 0, :],
                                 start=True, stop=False)
                nc.tensor.matmul(fp, H2_[1 - par][:, 1, :], wlin_t[:, 1, :],
                                 start=False, stop=False)
                nc.tensor.matmul(fp, eyer_t, y_[1 - par],
                                 start=False, stop=False)
                nc.tensor.matmul(fp, ones1_t, blin_t,
                                 start=False, stop=True)
                o_t = iop.tile([128, I], F32, tag="o")
                nc.scalar.copy(o_t, fp)
                nc.sync.dma_start(out=out_d[ds(r_o, C), :], in_=o_t)

            def body(r0, par, first=False):
                for k in range(GRP):
                    gstep(0, k, par)
                    if not first:
                        gstep(1, k, par)
                if debug:
                    # H1 is group g at rows r0; H2 is group g-1 (host shifts)
                    nc.sync.dma_start(
                        out=h1dbg_d[ds(r0, C), :, :],
                        in_=H1_[par].bitcast(BF16))
                    nc.sync.dma_start(
                        out=h2dbg_d[ds(r0, C), :, :],
                        in_=H2_[1 - par].bitcast(BF16))
                gx1(par)
                final(r0, par)
                bulk0(r0 + C, 1 - par)

            # prologue: group 0 inputs -> parity 0
            bulk0(0, 0)

            if unroll_all:
                for g in range(NG + 1):
                    body(g * C, g % 2, first=(g == 0))
            else:
                body(0, 0, first=True)
                with tc.For_i(C, (NG + 1) * C, 2 * C,
                              staggered_reset=True) as iv:
                    body(iv, 1)
                    body(iv + C, 0)

    nc.compile()
    return nc


def prep_weights(W_ih0, W_hh0, b_ih0, b_hh0, W_ih1, W_hh1, b_ih1, b_hh1,
                 W_lin, b_lin):
    """Host-side weight folding into gate-major bf16 layouts."""
    import ml_dtypes
    bf = ml_dtypes.bfloat16
    f = np.float32

    def whh_lay(W):  # [768, 256] -> [128, 6, 2, 128]
        return np.ascontiguousarray(
            W.reshape(NJ, 128, NI, 128).transpose(3, 0, 2, 1)).astype(bf)

    bias0 = (b_ih0 + np.concatenate([b_hh0[: 2 * H], np.zeros(H, f)])).astype(f)
    bias1 = (b_ih1 + np.concatenate([b_hh1[: 2 * H], np.zeros(H, f)])).astype(f)

    # wih0a: [65, 6, 128]; rows 0:64 = W_ih0^T gate-major, row 64 = bias0
    wih0a = np.zeros((I + 1, NJ, 128), f)
    wih0a[:I] = W_ih0.reshape(NJ, 128, I).transpose(2, 0, 1)
    wih0a[I] = bias0.reshape(NJ, 128)

    bc = lambda bh: np.broadcast_to(
        bh[2 * H:].reshape(2, 128).T[:, :, None], (128, 2, BL))

    return {
        "whh0": whh_lay(W_hh0),
        "whh1": whh_lay(W_hh1),
        "wih1": whh_lay(W_ih1),
        "wih0a": wih0a.astype(bf),
        "wlin": np.ascontiguousarray(
            W_lin.T.reshape(NI, 128, I).transpose(1, 0, 2)).astype(bf),
        "bias1": bias1.reshape(1, NJ, 128).astype(bf),
        "blin": b_lin.reshape(1, I).astype(bf),
        "bc0": np.ascontiguousarray(bc(b_hh0)).astype(bf),
        "bc1": np.ascontiguousarray(bc(b_hh1)).astype(bf),
        "eyef": np.eye(128, dtype=f),
        "eyeb": np.eye(128, dtype=bf),
        "eyer": np.eye(128, dtype=f),
    }


def prep_seq(a, T):
    """[BLc, T, I] f32 -> padded [RPAD_IN, I] rows (t*BLc+b order)."""
    BLc = a.shape[0]
    NG = T // GRP
    r = np.ascontiguousarray(a.transpose(1, 0, 2)).reshape(T * BLc, I)
    pad = np.zeros(((NG + 2) * GRP * BLc - T * BLc, I), np.float32)
    return np.concatenate([r, pad], axis=0)


def unprep_out(o, T):
    """[RPAD_OUT, I] -> [BL, T, I] (drop first pad group)."""
    o = o[C:].reshape(T, BL, I)
    return np.ascontiguousarray(o.transpose(1, 0, 2))


_NC_CACHE = {}


def kernel(z, y, W_ih0, W_hh0, b_ih0, b_hh0, W_ih1, W_hh1, b_ih1, b_hh1,
           W_lin, b_lin, _trace=False):
    """Full-input entry point: shards over 8 cores, returns full output."""
    from concourse.bass_utils import run_bass_kernel_spmd

    z = np.asarray(z, np.float32)
    y = np.asarray(y, np.float32)
    T = z.shape[1]
    if T not in _NC_CACHE:
        _NC_CACHE[T] = build_nc(T=T)
    nc = _NC_CACHE[T]

    wmaps = prep_weights(
        np.asarray(W_ih0), np.asarray(W_hh0), np.asarray(b_ih0),
        np.asarray(b_hh0), np.asarray(W_ih1), np.asarray(W_hh1),
        np.asarray(b_ih1), np.asarray(b_hh1), np.asarray(W_lin),
        np.asarray(b_lin))

    in_maps = []
    for cid in range(NCORES):
        sl = slice(cid * BL, (cid + 1) * BL)
        m = {"y": prep_seq(y[sl], T), "z": prep_seq(z[sl], T)}
        m.update(wmaps)
        in_maps.append(m)

    res = run_bass_kernel_spmd(nc, in_maps, list(range(NCORES)),
                               trace=_trace)
    outs = [unprep_out(res.results[cid]["out"], T) for cid in range(NCORES)]
    full = np.concatenate(outs, axis=0).astype(np.float32)
    if _trace:
        return full, res
    return full
